# revision 26
# baseline (speedup 1.0000x reference)
"""Causal self-attention Trainium2 kernel.

Full inputs -> full outputs. Data-parallel over batch across 8 NeuronCores
(16 batches per core), no collectives.

Per-core design (bf16 matmul operands, fp32 PSUM accumulate):
  - x is pre-transposed ON HOST to XT [C, tok] bf16 (DMA'd per group), so
    the kernel never spends PE cycles transposing activations.
  - Q^T/K^T [feature, tok] bf16: lhsT = w_attn^T chunk, rhs = XT; the 1/8
    score scale is folded into the Q weights/bias on the host; bias applied
    on eviction, alternating ACT (with bias) / DVE (tensor_scalar_add) to
    balance engine load.
  - V [tok, feature] bf16 with an interleaved ones column per head so row 64
    of the PV output is Z = sum_k P.
  - Scores per (batch, head) are computed transposed and PACKED [k=128, 384]
    with only TWO matmuls (no mask matmuls): cols 0:256 = k-tile0 x
    (q 0..255) in one N=256 matmul (shared lhsT), cols 256:384 = k-tile1 x
    (q 128..255). The fully-masked (k-tile1, q<128) quadrant is never
    computed. One ACT exp eviction -> P bf16, then the two diagonal
    quadrants are multiplied by a 0/1 triangular mask on DVE (cheaper than
    the old -1e30 identity-matmul pre-bias, which cost 2 PE matmuls/head).
  - PV: 3 N=65 matmuls -> O~ [128q, 130] (cols 64/129 = Z per q-tile).
  - Normalize per-partition (per-q) on Pool (normalize_recip), transpose
    On on PE, copy into the pair-stacked OT tile (head parity picks
    partitions 0:64 / 64:128), so projection runs with K=128 lhsT tiles.
  - Projection: 3 K=128 accumulating matmuls per 128-token tile; bias added
    on the DVE eviction (tensor_tensor add with a broadcast bias tile);
    one batched output DMA per group.
"""

import numpy as np
import ml_dtypes

import concourse.bass as bass
import concourse.bacc as bacc
import concourse.mybir as mybir
import concourse.tile as tile

N_CORES = 8
B, T, C = 128, 256, 384
H, HD = 6, 64
NB = B // N_CORES          # batches per core
TOK = NB * T               # tokens per core
G = 2                      # batches per group
NG = NB // G               # groups per core
GT = G * T                 # tokens per group (512)
NTT = GT // 128            # 128-token tiles per group (4)
F32 = mybir.dt.float32
BF16 = mybir.dt.bfloat16
AF = mybir.ActivationFunctionType
ALU = mybir.AluOpType


def _body(tc, xt_d, wat_d, wpt_d, bq_d, bk_d, beff_d, trim_d, identb_d,
          y_d):
    nc = tc.nc
    from contextlib import ExitStack

    ctx = ExitStack()
    with ctx:
        const = ctx.enter_context(tc.tile_pool(name="const", bufs=1))
        xt = ctx.enter_context(tc.tile_pool(name="xt", bufs=2))
        qkt = ctx.enter_context(tc.tile_pool(name="qkt", bufs=2))
        v65 = ctx.enter_context(tc.tile_pool(name="v65", bufs=2))
        pp = ctx.enter_context(tc.tile_pool(name="pp", bufs=6))
        oe = ctx.enter_context(tc.tile_pool(name="oe", bufs=6))
        onp = ctx.enter_context(tc.tile_pool(name="onp", bufs=4))
        ot = ctx.enter_context(tc.tile_pool(name="ot", bufs=2))
        yb = ctx.enter_context(tc.tile_pool(name="yb", bufs=2))
        mm_ps = ctx.enter_context(tc.tile_pool(name="mm_ps", bufs=2, space="PSUM"))
        s_ps = ctx.enter_context(tc.tile_pool(name="s_ps", bufs=2, space="PSUM"))
        o_ps = ctx.enter_context(tc.tile_pool(name="o_ps", bufs=2, space="PSUM"))

        dma = nc.sync.dma_start

        wat_sb = const.tile([128, 3, 3 * C], BF16, name="wat_sb")
        wpt_sb = const.tile([128, 3, C], BF16, name="wpt_sb")
        bq_sb = const.tile([128, 3], F32, name="bq_sb")
        bk_sb = const.tile([128, 3], F32, name="bk_sb")
        beff_sb = const.tile([128, C], F32, name="beff_sb")
        trim2_sb = const.tile([128, 2, 128], BF16, name="trim2_sb")
        identb_sb = const.tile([128, 128], BF16, name="identb_sb")

        dma(wat_sb[:], wat_d.ap().rearrange("(ct p) f -> p ct f", p=128))
        dma(wpt_sb[:], wpt_d.ap())
        dma(bq_sb[:], bq_d.ap())
        dma(bk_sb[:], bk_d.ap())
        dma(beff_sb[:], beff_d.ap())
        dma(trim2_sb[:], trim_d.ap())
        dma(identb_sb[:], identb_d.ap())

        xtv = xt_d.ap().rearrange("ct p (g t) -> g p ct t", t=GT)
        yv = y_d.ap().rearrange("(g tt p) c -> g p tt c", tt=NTT, p=128)

        # Tiles live across the gen(g+1) / attention(g) software pipeline.
        st = {}

        def gen_chunks(g):
            """QKV generation for group g as a list of emit-closures."""
            XT_sb = xt.tile([128, 3, GT], BF16, name=f"XT_{g}", tag="XT")
            QKT_sb = qkt.tile([128, 6, GT], BF16, name=f"QKT_{g}", tag="QKT")
            V65_sb = v65.tile([128, NTT, H * 65], BF16, name=f"V65_{g}",
                              tag="V65")
            st[g] = (QKT_sb, V65_sb)
            chunks = [lambda: dma(XT_sb[:], xtv[g])]

            def qkgen(ft):
                ps_qk = mm_ps.tile([128, 512], F32, name=f"psqk_{g}_{ft}",
                                   tag="mm")
                for ct in range(3):
                    nc.tensor.matmul(
                        ps_qk[:],
                        wat_sb[:, ct, 128 * ft:128 * (ft + 1)],
                        XT_sb[:, ct, :],
                        start=(ct == 0),
                        stop=(ct == 2),
                    )
                bias = bq_sb[:, ft:ft + 1] if ft < 3 else bk_sb[:, ft - 3:ft - 2]
                # split the eviction between ACT and DVE so neither engine
                # eats the whole 512-col copy in one wave
                nc.scalar.activation(QKT_sb[:, ft, 0:256], ps_qk[:, 0:256],
                                     AF.Identity, bias=bias)
                nc.vector.tensor_scalar_add(QKT_sb[:, ft, 256:512],
                                            ps_qk[:, 256:512], bias)

            def vgen(tt):
                ps_v = mm_ps.tile([128, 384], F32, name=f"psv_{g}_{tt}",
                                  tag="mm")
                for ct in range(3):
                    nc.tensor.matmul(
                        ps_v[:],
                        XT_sb[:, ct, 128 * tt:128 * (tt + 1)],
                        wat_sb[:, ct, 2 * C:3 * C],
                        start=(ct == 0),
                        stop=(ct == 2),
                    )
                v_view = V65_sb[:, tt, :].rearrange("p (h w) -> p h w", h=H)
                if tt % 2 == 0:
                    nc.vector.tensor_copy(
                        v_view[:, :, 0:64],
                        ps_v[:].rearrange("p (h w) -> p h w", h=H),
                    )
                else:
                    nc.scalar.copy(
                        v_view[:, :, 0:64],
                        ps_v[:].rearrange("p (h w) -> p h w", h=H),
                    )
                nc.gpsimd.memset(v_view[:, :, 64:65], 1.0)

            from functools import partial
            chunks += [partial(qkgen, ft) for ft in range(6)]
            chunks += [partial(vgen, tt) for tt in range(NTT)]
            return chunks

        def att_chunks(g):
            """Attention + projection for group g as a list of emit-closures.

            The per-head work is stage-split (scores -> mask -> PV ->
            normalize -> transpose) and emitted in waves so the PE never
            sits directly behind an ACT exp or a DVE mask.
            """
            QKT_sb, V65_sb = st[g]
            OT_sb = ot.tile([128, 3, GT], BF16, name=f"OT_{g}", tag="OT")
            Y_sb = yb.tile([128, NTT, C], F32, name=f"Y_{g}", tag="Y")
            hs = {}

            def scores(bl, h):
                q0 = 256 * bl
                ft, r0 = h // 2, 64 * (h % 2)
                hp, c0 = h // 2, 512 * (h % 2)
                KT = QKT_sb[r0:r0 + 64, 3 + ft, :]
                QT = QKT_sb[r0:r0 + 64, ft, :]

                if h % 2 == 0:
                    # one 2-bank PSUM tile per head pair; head-odd scores
                    # start at col 512 so each head stays within a bank
                    hs[(bl, hp, "s2")] = s_ps.tile(
                        [128, 1024], F32, name=f"pss_{g}_{bl}_{hp}", tag="s")
                ps_s = hs[(bl, hp, "s2")]
                # cols 0:256 = k-tile0 x (q 0:256), one matmul (shared lhsT)
                nc.tensor.matmul(
                    ps_s[:, c0:c0 + 256],
                    KT[:, q0:q0 + 128],
                    QT[:, q0:q0 + 256],
                    start=True, stop=True,
                )
                # cols 256:384 = k-tile1 x (q 128:256)
                nc.tensor.matmul(
                    ps_s[:, c0 + 256:c0 + 384],
                    KT[:, q0 + 128:q0 + 256],
                    QT[:, q0 + 128:q0 + 256],
                    start=True, stop=True,
                )

            def exp2(bl, hp):
                # one ACT exp for the whole head pair (strided PSUM read)
                ps_s = hs.pop((bl, hp, "s2"))
                P2_sb = pp.tile([128, 2, 384], BF16, name=f"P_{g}_{bl}_{hp}",
                                tag="P")
                sv = ps_s[:, :].rearrange("p (a b) -> p a b", a=2)[:, :, 0:384]
                nc.scalar.activation(P2_sb[:], sv, AF.Exp)
                hs[(bl, hp, "P")] = P2_sb

            def maskp(bl, h):
                # zero the upper-triangular (k > q) entries of the two
                # diagonal quadrants (cols 0:128 and 256:384) with a single
                # strided 0/1 mask multiply on DVE
                P2_sb = hs[(bl, h // 2, "P")]
                pq = P2_sb[:, h % 2, :].rearrange(
                    "p (a b) -> p a b", a=3)[:, 0:3:2, :]
                nc.vector.tensor_tensor(pq, pq, trim2_sb[:], op=ALU.mult)

            def pv(bl, h):
                hp, c0 = h // 2, 130 * (h % 2)
                P_sb = hs[(bl, hp, "P")][:, h % 2, :]
                vt0 = V65_sb[:, 2 * bl, 65 * h:65 * h + 65]
                vt1 = V65_sb[:, 2 * bl + 1, 65 * h:65 * h + 65]
                # O[q, f] with q on partitions; col 64/129 = Z; both heads
                # of a pair share one PSUM tile
                if h % 2 == 0:
                    hs[(bl, hp, "o2")] = o_ps.tile(
                        [128, 260], F32, name=f"pso_{g}_{bl}_{hp}", tag="o")
                else:
                    hs.pop((bl, hp, "P"))
                ps_o = hs[(bl, hp, "o2")]
                nc.tensor.matmul(ps_o[:, c0:c0 + 65], P_sb[:, 0:128], vt0,
                                 start=True, stop=True)
                nc.tensor.matmul(ps_o[:, c0 + 65:c0 + 130], P_sb[:, 128:256],
                                 vt0, start=True, stop=False)
                nc.tensor.matmul(ps_o[:, c0 + 65:c0 + 130], P_sb[:, 256:384],
                                 vt1, start=False, stop=True)

            def norma(bl, hp):
                # one paired PSUM->SBUF eviction for both heads
                ps_o = hs.pop((bl, hp, "o2"))
                Oq_sb = oe.tile([128, 260], F32, name=f"Oq_{g}_{bl}_{hp}",
                                tag="Oq")
                if (bl * 3 + hp) % 2 == 0:
                    nc.vector.tensor_copy(Oq_sb[:], ps_o[:])
                else:
                    nc.scalar.copy(Oq_sb[:], ps_o[:])
                hs[(bl, hp, "q")] = Oq_sb

            def normb(bl, hp):
                Oq_sb = hs.pop((bl, hp, "q"))
                # per-partition (per-q) normalize on Pool into the head-PAIR
                # tiles: OnP_qt [128 q, 128] = [head-even f | head-odd f]
                OnP0 = onp.tile([128, 128], BF16, name=f"OnP0_{g}_{bl}_{hp}",
                                tag="OnP")
                OnP1 = onp.tile([128, 128], BF16, name=f"OnP1_{g}_{bl}_{hp}",
                                tag="OnP")
                for par in range(2):
                    b0 = 130 * par
                    fcol = 64 * par
                    nc.gpsimd.normalize_recip(
                        OnP0[:, fcol:fcol + 64], Oq_sb[:, b0:b0 + 64],
                        Oq_sb[:, b0 + 64:b0 + 65])
                    nc.gpsimd.normalize_recip(
                        OnP1[:, fcol:fcol + 64], Oq_sb[:, b0 + 65:b0 + 129],
                        Oq_sb[:, b0 + 129:b0 + 130])
                hs[(bl, hp, "np0")] = OnP0
                hs[(bl, hp, "np1")] = OnP1

            def otrp(bl, hp):
                # PE-transpose each pair tile straight into the projection
                # layout: OT[0:128, hp, qtile] = OnP_qt.T (f-pair rows); one
                # full-width copy per q-tile, split DVE / ACT
                q0 = 256 * bl
                for qt in range(2):
                    OnP = hs.pop((bl, hp, "np0" if qt == 0 else "np1"))
                    ps_t = o_ps.tile([128, 128], BF16,
                                     name=f"ptr_{g}_{bl}_{hp}_{qt}",
                                     tag="o")
                    nc.tensor.transpose(ps_t[:], OnP[:], identb_sb[:])
                    dst = OT_sb[:, hp, q0 + 128 * qt:q0 + 128 * (qt + 1)]
                    if qt == 0:
                        nc.vector.tensor_copy(dst, ps_t[:])
                    else:
                        nc.scalar.copy(dst, ps_t[:])

            def proj(tt):
                ps_y = mm_ps.tile([128, 384], F32, name=f"psy_{g}_{tt}",
                                  tag="mm")
                for hp in range(3):
                    nc.tensor.matmul(
                        ps_y[:],
                        OT_sb[:, hp, 128 * tt:128 * (tt + 1)],
                        wpt_sb[:, hp, :],
                        start=(hp == 0),
                        stop=(hp == 2),
                    )
                nc.vector.tensor_tensor(Y_sb[:, tt, :], ps_y[:], beff_sb[:],
                                        op=ALU.add)

            from functools import partial
            heads = [(bl, h) for bl in range(G) for h in range(H)]
            nh = len(heads)

            def pair_stage(fn, j, wave):
                if 0 <= j < nh and heads[j][1] % 2 == 1:
                    wave.append(partial(fn, heads[j][0], heads[j][1] // 2))

            chunks = []
            for i in range(nh + 8):
                wave = []
                if i < nh:
                    wave.append(partial(scores, *heads[i]))
                pair_stage(exp2, i - 1, wave)
                if 0 <= i - 3 < nh:
                    wave.append(partial(maskp, *heads[i - 3]))
                if 0 <= i - 4 < nh:
                    wave.append(partial(pv, *heads[i - 4]))
                pair_stage(norma, i - 5, wave)
                pair_stage(normb, i - 6, wave)
                pair_stage(otrp, i - 7, wave)
                j = i - 7
                if 0 <= j < nh and heads[j] == (heads[j][0], H - 1):
                    # this wave completed otrp(bl, 2): OT q-block bl is
                    # done -> project + ship it while later heads drain
                    bl = heads[j][0]
                    wave.append(partial(proj, 2 * bl))
                    wave.append(partial(proj, 2 * bl + 1))
                    wave.append(lambda b=bl: dma(
                        yv[g][:, 2 * b:2 * b + 2, :],
                        Y_sb[:, 2 * b:2 * b + 2, :]))
                chunks.append(lambda w=wave: [f() for f in w])
            return chunks

        # software pipeline: interleave gen(g+1) into the TAIL of the
        # attention(g) chunk list, where the norm-pipeline drain leaves the
        # PE underfed (the dense scores/pv waves keep it busy on their own)
        for c in gen_chunks(0):
            c()
        for g in range(NG):
            att = att_chunks(g)
            gen = gen_chunks(g + 1) if g + 1 < NG else []
            ofs = max(0, len(att) - len(gen))
            for i in range(len(att)):
                att[i]()
                j = i - ofs
                if 0 <= j < len(gen):
                    gen[j]()
            for j in range(len(att) - ofs, len(gen)):
                gen[j]()


_CACHE = {}


def _build_nc():
    if "nc" in _CACHE:
        return _CACHE["nc"]
    nc = bacc.Bacc("TRN2", target_bir_lowering=False, debug=False,
                   num_devices=N_CORES)
    xt_d = nc.dram_tensor("xt", [3, 128, TOK], BF16, kind="ExternalInput")
    wat_d = nc.dram_tensor("w_attnT", [C, 3 * C], BF16, kind="ExternalInput")
    wpt_d = nc.dram_tensor("w_projT", [128, 3, C], BF16, kind="ExternalInput")
    bq_d = nc.dram_tensor("bq", [128, 3], F32, kind="ExternalInput")
    bk_d = nc.dram_tensor("bk", [128, 3], F32, kind="ExternalInput")
    beff_d = nc.dram_tensor("beff", [128, C], F32, kind="ExternalInput")
    trim_d = nc.dram_tensor("trim", [128, 2, 128], BF16, kind="ExternalInput")
    identb_d = nc.dram_tensor("identb", [128, 128], BF16, kind="ExternalInput")
    y_d = nc.dram_tensor("y", [TOK, C], F32, kind="ExternalOutput")

    with tile.TileContext(nc) as tc:
        _body(tc, xt_d, wat_d, wpt_d, bq_d, bk_d, beff_d, trim_d, identb_d,
              y_d)
    nc.compile()
    _CACHE["nc"] = nc
    return nc


def _host_inputs(x, w_attn, b_attn, w_proj, b_proj):
    """Build the per-core input maps (host-side prep of weights/constants)."""
    bf16 = ml_dtypes.bfloat16
    w_attnT = np.ascontiguousarray(w_attn.T).astype(np.float32)    # [C, 3C]
    w_attnT[:, :C] *= 0.125          # fold score scale into Q weights
    # w_projT per head pair: wpt[p, hp, of] = w_proj[of, 128*hp + p]
    wpt = np.ascontiguousarray(
        w_proj.T.reshape(3, 128, C).transpose(1, 0, 2))
    bq = np.ascontiguousarray((0.125 * b_attn[:C]).reshape(3, 128).T)
    bk = np.ascontiguousarray(b_attn[C:2 * C].reshape(3, 128).T)
    b_eff = np.broadcast_to(b_proj + w_proj @ b_attn[2 * C:], (128, C))

    # 0/1 lower-triangular (k <= q) mask, doubled for the two diagonal
    # quadrants masked in one strided op
    p = np.arange(128)[:, None]
    j = np.arange(128)[None, :]
    mask = (p <= j).astype(np.float32)
    mask2 = np.stack([mask, mask], axis=1)          # [128, 2, 128]

    common = {
        "w_attnT": w_attnT.astype(bf16),
        "w_projT": wpt.astype(bf16),
        "bq": bq.astype(np.float32),
        "bk": bk.astype(np.float32),
        "beff": np.ascontiguousarray(b_eff).astype(np.float32),
        "trim": np.ascontiguousarray(mask2).astype(bf16),
        "identb": np.eye(128, dtype=np.float32).astype(bf16),
    }
    # host-side transpose: x [B,T,C] -> per-core XT [3, 128, TOK]
    xs = x.reshape(N_CORES, TOK, C)
    in_maps = []
    for c in range(N_CORES):
        xt_c = np.ascontiguousarray(xs[c].T.reshape(3, 128, TOK))
        m = dict(common)
        m["xt"] = xt_c.astype(bf16)
        in_maps.append(m)
    return in_maps


def kernel(x, w_attn, b_attn, w_proj, b_proj):
    from concourse.bass_utils import run_bass_kernel_spmd

    x = np.asarray(x, dtype=np.float32)
    w_attn = np.asarray(w_attn, dtype=np.float32)
    b_attn = np.asarray(b_attn, dtype=np.float32)
    w_proj = np.asarray(w_proj, dtype=np.float32)
    b_proj = np.asarray(b_proj, dtype=np.float32)

    nc = _build_nc()
    in_maps = _host_inputs(x, w_attn, b_attn, w_proj, b_proj)
    res = run_bass_kernel_spmd(nc, in_maps, core_ids=list(range(N_CORES)))
    y = np.stack([res.results[c]["y"] for c in range(N_CORES)])
    return y.reshape(B, T, C)


# revision 27
# speedup vs baseline: 1.1550x; 1.1550x over previous
"""Causal self-attention Trainium2 kernel.

Full inputs -> full outputs. Data-parallel over batch across 8 NeuronCores
(16 batches per core), no collectives.

Per-core design (bf16 matmul operands, fp32 PSUM accumulate):
  - x is pre-transposed ON HOST to XT [C, tok] bf16 (DMA'd per group), so
    the kernel never spends PE cycles transposing activations.
  - Q^T/K^T [feature, tok] bf16: lhsT = w_attn^T chunk, rhs = XT; the 1/8
    score scale is folded into the Q weights/bias on the host; bias applied
    on eviction, alternating ACT (with bias) / DVE (tensor_scalar_add) to
    balance engine load.
  - V [tok, feature] bf16 with an interleaved ones column per head so row 64
    of the PV output is Z = sum_k P.
  - Scores per (batch, head) are computed transposed and PACKED [k=128, 384]
    with only TWO matmuls (no mask matmuls): cols 0:256 = k-tile0 x
    (q 0..255) in one N=256 matmul (shared lhsT), cols 256:384 = k-tile1 x
    (q 128..255). The fully-masked (k-tile1, q<128) quadrant is never
    computed. One ACT exp eviction -> P bf16, then the two diagonal
    quadrants are multiplied by a 0/1 triangular mask on DVE (cheaper than
    the old -1e30 identity-matmul pre-bias, which cost 2 PE matmuls/head).
  - PV: 3 N=65 matmuls -> O~ [128q, 130] (cols 64/129 = Z per q-tile).
  - Normalize per-partition (per-q) on Pool (normalize_recip), transpose
    On on PE, copy into the pair-stacked OT tile (head parity picks
    partitions 0:64 / 64:128), so projection runs with K=128 lhsT tiles.
  - Projection: 3 K=128 accumulating matmuls per 128-token tile; bias added
    on the DVE eviction (tensor_tensor add with a broadcast bias tile);
    one batched output DMA per group.
"""

import numpy as np
import ml_dtypes

import concourse.bass as bass
import concourse.bacc as bacc
import concourse.mybir as mybir
import concourse.tile as tile

N_CORES = 8
B, T, C = 128, 256, 384
H, HD = 6, 64
NB = B // N_CORES          # batches per core
TOK = NB * T               # tokens per core
G = 2                      # batches per group
NG = NB // G               # groups per core
GT = G * T                 # tokens per group (512)
NTT = GT // 128            # 128-token tiles per group (4)
F32 = mybir.dt.float32
BF16 = mybir.dt.bfloat16
AF = mybir.ActivationFunctionType
ALU = mybir.AluOpType


def _body(tc, xt_d, wat_d, wpt_d, bq_d, bk_d, beff_d, trim_d, identb_d,
          y_d):
    nc = tc.nc
    from contextlib import ExitStack

    ctx = ExitStack()
    with ctx:
        const = ctx.enter_context(tc.tile_pool(name="const", bufs=1))
        xt = ctx.enter_context(tc.tile_pool(name="xt", bufs=2))
        qkt = ctx.enter_context(tc.tile_pool(name="qkt", bufs=2))
        v65 = ctx.enter_context(tc.tile_pool(name="v65", bufs=2))
        pp = ctx.enter_context(tc.tile_pool(name="pp", bufs=6))
        oe = ctx.enter_context(tc.tile_pool(name="oe", bufs=6))
        onp = ctx.enter_context(tc.tile_pool(name="onp", bufs=4))
        ot = ctx.enter_context(tc.tile_pool(name="ot", bufs=2))
        yb = ctx.enter_context(tc.tile_pool(name="yb", bufs=2))
        mm_ps = ctx.enter_context(tc.tile_pool(name="mm_ps", bufs=2, space="PSUM"))
        s_ps = ctx.enter_context(tc.tile_pool(name="s_ps", bufs=2, space="PSUM"))
        o_ps = ctx.enter_context(tc.tile_pool(name="o_ps", bufs=2, space="PSUM"))

        dma = nc.sync.dma_start

        wat_sb = const.tile([128, 3, 3 * C], BF16, name="wat_sb")
        wpt_sb = const.tile([128, 3, C], BF16, name="wpt_sb")
        bq_sb = const.tile([128, 3], F32, name="bq_sb")
        bk_sb = const.tile([128, 3], F32, name="bk_sb")
        beff_sb = const.tile([128, C], F32, name="beff_sb")
        trim2_sb = const.tile([128, 2, 128], BF16, name="trim2_sb")
        identb_sb = const.tile([128, 128], BF16, name="identb_sb")

        dma(wat_sb[:], wat_d.ap().rearrange("(ct p) f -> p ct f", p=128))
        dma(wpt_sb[:], wpt_d.ap())
        dma(bq_sb[:], bq_d.ap())
        dma(bk_sb[:], bk_d.ap())
        dma(beff_sb[:], beff_d.ap())
        dma(trim2_sb[:], trim_d.ap())
        dma(identb_sb[:], identb_d.ap())

        xtv = xt_d.ap().rearrange("ct p (g t) -> g p ct t", t=GT)
        yv = y_d.ap().rearrange("(g tt p) c -> g p tt c", tt=NTT, p=128)

        # Tiles live across the gen(g+1) / attention(g) software pipeline.
        st = {}

        def gen_chunks(g):
            """QKV generation for group g as a list of emit-closures."""
            XT_sb = xt.tile([128, 3, GT], BF16, name=f"XT_{g}", tag="XT")
            QKT_sb = qkt.tile([128, 6, GT], BF16, name=f"QKT_{g}", tag="QKT")
            V65_sb = v65.tile([128, NTT, H * 65], BF16, name=f"V65_{g}",
                              tag="V65")
            st[g] = (QKT_sb, V65_sb)
            chunks = [lambda: dma(XT_sb[:], xtv[g])]

            def qkgen(ft):
                ps_qk = mm_ps.tile([128, 512], F32, name=f"psqk_{g}_{ft}",
                                   tag="mm")
                for ct in range(3):
                    nc.tensor.matmul(
                        ps_qk[:],
                        wat_sb[:, ct, 128 * ft:128 * (ft + 1)],
                        XT_sb[:, ct, :],
                        start=(ct == 0),
                        stop=(ct == 2),
                    )
                bias = bq_sb[:, ft:ft + 1] if ft < 3 else bk_sb[:, ft - 3:ft - 2]
                # split the eviction between ACT and DVE so neither engine
                # eats the whole 512-col copy in one wave
                nc.scalar.activation(QKT_sb[:, ft, 0:256], ps_qk[:, 0:256],
                                     AF.Identity, bias=bias)
                nc.vector.tensor_scalar_add(QKT_sb[:, ft, 256:512],
                                            ps_qk[:, 256:512], bias)

            def vgen(tt):
                ps_v = mm_ps.tile([128, 384], F32, name=f"psv_{g}_{tt}",
                                  tag="mm")
                for ct in range(3):
                    nc.tensor.matmul(
                        ps_v[:],
                        XT_sb[:, ct, 128 * tt:128 * (tt + 1)],
                        wat_sb[:, ct, 2 * C:3 * C],
                        start=(ct == 0),
                        stop=(ct == 2),
                    )
                v_view = V65_sb[:, tt, :].rearrange("p (h w) -> p h w", h=H)
                if tt % 2 == 0:
                    nc.vector.tensor_copy(
                        v_view[:, :, 0:64],
                        ps_v[:].rearrange("p (h w) -> p h w", h=H),
                    )
                else:
                    nc.scalar.copy(
                        v_view[:, :, 0:64],
                        ps_v[:].rearrange("p (h w) -> p h w", h=H),
                    )
                nc.gpsimd.memset(v_view[:, :, 64:65], 1.0)

            from functools import partial
            chunks += [partial(qkgen, ft) for ft in range(6)]
            chunks += [partial(vgen, tt) for tt in range(NTT)]
            return chunks

        def att_chunks(g):
            """Attention + projection for group g as a list of emit-closures.

            The per-head work is stage-split (scores -> mask -> PV ->
            normalize -> transpose) and emitted in waves so the PE never
            sits directly behind an ACT exp or a DVE mask.
            """
            QKT_sb, V65_sb = st[g]
            OT_sb = ot.tile([128, 3, GT], BF16, name=f"OT_{g}", tag="OT")
            Y_sb = yb.tile([128, NTT, C], F32, name=f"Y_{g}", tag="Y")
            hs = {}

            def scores(bl, h):
                q0 = 256 * bl
                ft, r0 = h // 2, 64 * (h % 2)
                hp, c0 = h // 2, 512 * (h % 2)
                KT = QKT_sb[r0:r0 + 64, 3 + ft, :]
                QT = QKT_sb[r0:r0 + 64, ft, :]

                if h % 2 == 0:
                    # one 2-bank PSUM tile per head pair; head-odd scores
                    # start at col 512 so each head stays within a bank
                    hs[(bl, hp, "s2")] = s_ps.tile(
                        [128, 1024], F32, name=f"pss_{g}_{bl}_{hp}", tag="s")
                ps_s = hs[(bl, hp, "s2")]
                # cols 0:256 = k-tile0 x (q 0:256), one matmul (shared lhsT)
                nc.tensor.matmul(
                    ps_s[:, c0:c0 + 256],
                    KT[:, q0:q0 + 128],
                    QT[:, q0:q0 + 256],
                    start=True, stop=True,
                )
                # cols 256:384 = k-tile1 x (q 128:256)
                nc.tensor.matmul(
                    ps_s[:, c0 + 256:c0 + 384],
                    KT[:, q0 + 128:q0 + 256],
                    QT[:, q0 + 128:q0 + 256],
                    start=True, stop=True,
                )

            def exp2(bl, hp):
                # one ACT exp for the whole head pair (strided PSUM read)
                ps_s = hs.pop((bl, hp, "s2"))
                P2_sb = pp.tile([128, 2, 384], BF16, name=f"P_{g}_{bl}_{hp}",
                                tag="P")
                sv = ps_s[:, :].rearrange("p (a b) -> p a b", a=2)[:, :, 0:384]
                nc.scalar.activation(P2_sb[:], sv, AF.Exp)
                hs[(bl, hp, "P")] = P2_sb

            def maskp(bl, h):
                # zero the upper-triangular (k > q) entries of the two
                # diagonal quadrants (cols 0:128 and 256:384) with a single
                # strided 0/1 mask multiply on DVE
                P2_sb = hs[(bl, h // 2, "P")]
                pq = P2_sb[:, h % 2, :].rearrange(
                    "p (a b) -> p a b", a=3)[:, 0:3:2, :]
                nc.vector.tensor_tensor(pq, pq, trim2_sb[:], op=ALU.mult)

            def pv(bl, h):
                hp, c0 = h // 2, 130 * (h % 2)
                P_sb = hs[(bl, hp, "P")][:, h % 2, :]
                vt0 = V65_sb[:, 2 * bl, 65 * h:65 * h + 65]
                vt1 = V65_sb[:, 2 * bl + 1, 65 * h:65 * h + 65]
                # O[q, f] with q on partitions; col 64/129 = Z; both heads
                # of a pair share one PSUM tile
                if h % 2 == 0:
                    hs[(bl, hp, "o2")] = o_ps.tile(
                        [128, 260], F32, name=f"pso_{g}_{bl}_{hp}", tag="o")
                else:
                    hs.pop((bl, hp, "P"))
                ps_o = hs[(bl, hp, "o2")]
                nc.tensor.matmul(ps_o[:, c0:c0 + 65], P_sb[:, 0:128], vt0,
                                 start=True, stop=True)
                nc.tensor.matmul(ps_o[:, c0 + 65:c0 + 130], P_sb[:, 128:256],
                                 vt0, start=True, stop=False)
                nc.tensor.matmul(ps_o[:, c0 + 65:c0 + 130], P_sb[:, 256:384],
                                 vt1, start=False, stop=True)

            def norma(bl, hp):
                # one paired PSUM->SBUF eviction for both heads
                ps_o = hs.pop((bl, hp, "o2"))
                Oq_sb = oe.tile([128, 260], F32, name=f"Oq_{g}_{bl}_{hp}",
                                tag="Oq")
                if (bl * 3 + hp) % 2 == 0:
                    nc.vector.tensor_copy(Oq_sb[:], ps_o[:])
                else:
                    nc.scalar.copy(Oq_sb[:], ps_o[:])
                hs[(bl, hp, "q")] = Oq_sb

            def normb(bl, hp):
                Oq_sb = hs.pop((bl, hp, "q"))
                # per-partition (per-q) normalize on Pool into the head-PAIR
                # tiles: OnP_qt [128 q, 128] = [head-even f | head-odd f]
                OnP0 = onp.tile([128, 128], BF16, name=f"OnP0_{g}_{bl}_{hp}",
                                tag="OnP")
                OnP1 = onp.tile([128, 128], BF16, name=f"OnP1_{g}_{bl}_{hp}",
                                tag="OnP")
                for par in range(2):
                    b0 = 130 * par
                    fcol = 64 * par
                    nc.gpsimd.normalize_recip(
                        OnP0[:, fcol:fcol + 64], Oq_sb[:, b0:b0 + 64],
                        Oq_sb[:, b0 + 64:b0 + 65])
                    nc.gpsimd.normalize_recip(
                        OnP1[:, fcol:fcol + 64], Oq_sb[:, b0 + 65:b0 + 129],
                        Oq_sb[:, b0 + 129:b0 + 130])
                hs[(bl, hp, "np0")] = OnP0
                hs[(bl, hp, "np1")] = OnP1

            def otrp(bl, hp):
                # PE-transpose each pair tile straight into the projection
                # layout: OT[0:128, hp, qtile] = OnP_qt.T (f-pair rows); one
                # full-width copy per q-tile, split DVE / ACT
                q0 = 256 * bl
                for qt in range(2):
                    OnP = hs.pop((bl, hp, "np0" if qt == 0 else "np1"))
                    ps_t = o_ps.tile([128, 128], BF16,
                                     name=f"ptr_{g}_{bl}_{hp}_{qt}",
                                     tag="o")
                    nc.tensor.transpose(ps_t[:], OnP[:], identb_sb[:])
                    dst = OT_sb[:, hp, q0 + 128 * qt:q0 + 128 * (qt + 1)]
                    if qt == 0:
                        nc.vector.tensor_copy(dst, ps_t[:])
                    else:
                        nc.scalar.copy(dst, ps_t[:])

            def proj(tt):
                ps_y = mm_ps.tile([128, 384], F32, name=f"psy_{g}_{tt}",
                                  tag="mm")
                for hp in range(3):
                    nc.tensor.matmul(
                        ps_y[:],
                        OT_sb[:, hp, 128 * tt:128 * (tt + 1)],
                        wpt_sb[:, hp, :],
                        start=(hp == 0),
                        stop=(hp == 2),
                    )
                nc.vector.tensor_tensor(Y_sb[:, tt, :], ps_y[:], beff_sb[:],
                                        op=ALU.add)

            from functools import partial
            heads = [(bl, h) for bl in range(G) for h in range(H)]
            nh = len(heads)

            def pair_stage(fn, j, wave):
                if 0 <= j < nh and heads[j][1] % 2 == 1:
                    wave.append(partial(fn, heads[j][0], heads[j][1] // 2))

            chunks = []
            for i in range(nh + 8):
                wave = []
                if i < nh:
                    wave.append(partial(scores, *heads[i]))
                pair_stage(exp2, i - 1, wave)
                if 0 <= i - 3 < nh:
                    wave.append(partial(maskp, *heads[i - 3]))
                if 0 <= i - 4 < nh:
                    wave.append(partial(pv, *heads[i - 4]))
                pair_stage(norma, i - 5, wave)
                pair_stage(normb, i - 6, wave)
                pair_stage(otrp, i - 7, wave)
                j = i - 7
                if 0 <= j < nh and heads[j] == (heads[j][0], H - 1):
                    # this wave completed otrp(bl, 2): OT q-block bl is
                    # done -> project + ship it while later heads drain
                    bl = heads[j][0]
                    wave.append(partial(proj, 2 * bl))
                    wave.append(partial(proj, 2 * bl + 1))
                    wave.append(lambda b=bl: dma(
                        yv[g][:, 2 * b:2 * b + 2, :],
                        Y_sb[:, 2 * b:2 * b + 2, :]))
                chunks.append(lambda w=wave: [f() for f in w])
            return chunks

        # software pipeline: interleave gen(g+1) between attention(g) chunks
        for c in gen_chunks(0):
            c()
        for g in range(NG):
            att = att_chunks(g)
            gen = gen_chunks(g + 1) if g + 1 < NG else []
            n = max(len(att), len(gen))
            for i in range(n):
                if i < len(att):
                    att[i]()
                if i < len(gen):
                    gen[i]()


_CACHE = {}


def _build_nc():
    if "nc" in _CACHE:
        return _CACHE["nc"]
    nc = bacc.Bacc("TRN2", target_bir_lowering=False, debug=False,
                   num_devices=N_CORES)
    xt_d = nc.dram_tensor("xt", [3, 128, TOK], BF16, kind="ExternalInput")
    wat_d = nc.dram_tensor("w_attnT", [C, 3 * C], BF16, kind="ExternalInput")
    wpt_d = nc.dram_tensor("w_projT", [128, 3, C], BF16, kind="ExternalInput")
    bq_d = nc.dram_tensor("bq", [128, 3], F32, kind="ExternalInput")
    bk_d = nc.dram_tensor("bk", [128, 3], F32, kind="ExternalInput")
    beff_d = nc.dram_tensor("beff", [128, C], F32, kind="ExternalInput")
    trim_d = nc.dram_tensor("trim", [128, 2, 128], BF16, kind="ExternalInput")
    identb_d = nc.dram_tensor("identb", [128, 128], BF16, kind="ExternalInput")
    y_d = nc.dram_tensor("y", [TOK, C], F32, kind="ExternalOutput")

    with tile.TileContext(nc) as tc:
        _body(tc, xt_d, wat_d, wpt_d, bq_d, bk_d, beff_d, trim_d, identb_d,
              y_d)
    nc.compile()
    _CACHE["nc"] = nc
    return nc


def _host_inputs(x, w_attn, b_attn, w_proj, b_proj):
    """Build the per-core input maps (host-side prep of weights/constants)."""
    bf16 = ml_dtypes.bfloat16
    w_attnT = np.ascontiguousarray(w_attn.T).astype(np.float32)    # [C, 3C]
    w_attnT[:, :C] *= 0.125          # fold score scale into Q weights
    # w_projT per head pair: wpt[p, hp, of] = w_proj[of, 128*hp + p]
    wpt = np.ascontiguousarray(
        w_proj.T.reshape(3, 128, C).transpose(1, 0, 2))
    bq = np.ascontiguousarray((0.125 * b_attn[:C]).reshape(3, 128).T)
    bk = np.ascontiguousarray(b_attn[C:2 * C].reshape(3, 128).T)
    b_eff = np.broadcast_to(b_proj + w_proj @ b_attn[2 * C:], (128, C))

    # 0/1 lower-triangular (k <= q) mask, doubled for the two diagonal
    # quadrants masked in one strided op
    p = np.arange(128)[:, None]
    j = np.arange(128)[None, :]
    mask = (p <= j).astype(np.float32)
    mask2 = np.stack([mask, mask], axis=1)          # [128, 2, 128]

    common = {
        "w_attnT": w_attnT.astype(bf16),
        "w_projT": wpt.astype(bf16),
        "bq": bq.astype(np.float32),
        "bk": bk.astype(np.float32),
        "beff": np.ascontiguousarray(b_eff).astype(np.float32),
        "trim": np.ascontiguousarray(mask2).astype(bf16),
        "identb": np.eye(128, dtype=np.float32).astype(bf16),
    }
    # host-side transpose: x [B,T,C] -> per-core XT [3, 128, TOK]
    xs = x.reshape(N_CORES, TOK, C)
    in_maps = []
    for c in range(N_CORES):
        xt_c = np.ascontiguousarray(xs[c].T.reshape(3, 128, TOK))
        m = dict(common)
        m["xt"] = xt_c.astype(bf16)
        in_maps.append(m)
    return in_maps


def kernel(x, w_attn, b_attn, w_proj, b_proj):
    from concourse.bass_utils import run_bass_kernel_spmd

    x = np.asarray(x, dtype=np.float32)
    w_attn = np.asarray(w_attn, dtype=np.float32)
    b_attn = np.asarray(b_attn, dtype=np.float32)
    w_proj = np.asarray(w_proj, dtype=np.float32)
    b_proj = np.asarray(b_proj, dtype=np.float32)

    nc = _build_nc()
    in_maps = _host_inputs(x, w_attn, b_attn, w_proj, b_proj)
    res = run_bass_kernel_spmd(nc, in_maps, core_ids=list(range(N_CORES)))
    y = np.stack([res.results[c]["y"] for c in range(N_CORES)])
    return y.reshape(B, T, C)


# revision 28
# speedup vs baseline: 1.1700x; 1.0130x over previous
"""Causal self-attention Trainium2 kernel.

Full inputs -> full outputs. Data-parallel over batch across 8 NeuronCores
(16 batches per core), no collectives.

Per-core design (bf16 matmul operands, fp32 PSUM accumulate):
  - x is pre-transposed ON HOST to XT [C, tok] bf16 (DMA'd per group), so
    the kernel never spends PE cycles transposing activations.
  - Q^T/K^T [feature, tok] bf16: lhsT = w_attn^T chunk, rhs = XT; the 1/8
    score scale is folded into the Q weights/bias on the host; bias applied
    on eviction, alternating ACT (with bias) / DVE (tensor_scalar_add) to
    balance engine load.
  - V [tok, feature] bf16 with an interleaved ones column per head so row 64
    of the PV output is Z = sum_k P.
  - Scores per (batch, head) are computed transposed and PACKED [k=128, 384]
    with only TWO matmuls (no mask matmuls): cols 0:256 = k-tile0 x
    (q 0..255) in one N=256 matmul (shared lhsT), cols 256:384 = k-tile1 x
    (q 128..255). The fully-masked (k-tile1, q<128) quadrant is never
    computed. One ACT exp eviction -> P bf16, then the two diagonal
    quadrants are multiplied by a 0/1 triangular mask on DVE (cheaper than
    the old -1e30 identity-matmul pre-bias, which cost 2 PE matmuls/head).
  - PV: 3 N=65 matmuls -> O~ [128q, 130] (cols 64/129 = Z per q-tile).
  - Normalize per-partition (per-q) on Pool (normalize_recip), transpose
    On on PE, copy into the pair-stacked OT tile (head parity picks
    partitions 0:64 / 64:128), so projection runs with K=128 lhsT tiles.
  - Projection: 3 K=128 accumulating matmuls per 128-token tile; bias added
    on the DVE eviction (tensor_tensor add with a broadcast bias tile);
    one batched output DMA per group.
"""

import numpy as np
import ml_dtypes

import concourse.bass as bass
import concourse.bacc as bacc
import concourse.mybir as mybir
import concourse.tile as tile

N_CORES = 8
B, T, C = 128, 256, 384
H, HD = 6, 64
NB = B // N_CORES          # batches per core
TOK = NB * T               # tokens per core
G = 2                      # batches per group
NG = NB // G               # groups per core
GT = G * T                 # tokens per group (512)
NTT = GT // 128            # 128-token tiles per group (4)
F32 = mybir.dt.float32
BF16 = mybir.dt.bfloat16
AF = mybir.ActivationFunctionType
ALU = mybir.AluOpType


def _body(tc, xt_d, wat_d, wpt_d, bq_d, bk_d, beff_d, trim_d, identb_d,
          y_d):
    nc = tc.nc
    from contextlib import ExitStack

    ctx = ExitStack()
    with ctx:
        const = ctx.enter_context(tc.tile_pool(name="const", bufs=1))
        xt = ctx.enter_context(tc.tile_pool(name="xt", bufs=2))
        qkt = ctx.enter_context(tc.tile_pool(name="qkt", bufs=2))
        v65 = ctx.enter_context(tc.tile_pool(name="v65", bufs=2))
        pp = ctx.enter_context(tc.tile_pool(name="pp", bufs=6))
        oe = ctx.enter_context(tc.tile_pool(name="oe", bufs=6))
        onp = ctx.enter_context(tc.tile_pool(name="onp", bufs=4))
        ot = ctx.enter_context(tc.tile_pool(name="ot", bufs=2))
        yb = ctx.enter_context(tc.tile_pool(name="yb", bufs=2))
        mm_ps = ctx.enter_context(tc.tile_pool(name="mm_ps", bufs=2, space="PSUM"))
        s_ps = ctx.enter_context(tc.tile_pool(name="s_ps", bufs=2, space="PSUM"))
        o_ps = ctx.enter_context(tc.tile_pool(name="o_ps", bufs=2, space="PSUM"))

        dma = nc.sync.dma_start

        wat_sb = const.tile([128, 3, 3 * C], BF16, name="wat_sb")
        wpt_sb = const.tile([128, 3, C], BF16, name="wpt_sb")
        bq_sb = const.tile([128, 3], F32, name="bq_sb")
        bk_sb = const.tile([128, 3], F32, name="bk_sb")
        beff_sb = const.tile([128, C], F32, name="beff_sb")
        trim2_sb = const.tile([128, 2, 128], BF16, name="trim2_sb")
        identb_sb = const.tile([128, 128], BF16, name="identb_sb")

        dma(wat_sb[:], wat_d.ap().rearrange("(ct p) f -> p ct f", p=128))
        dma(wpt_sb[:], wpt_d.ap())
        dma(bq_sb[:], bq_d.ap())
        dma(bk_sb[:], bk_d.ap())
        dma(beff_sb[:], beff_d.ap())
        dma(trim2_sb[:], trim_d.ap())
        dma(identb_sb[:], identb_d.ap())

        xtv = xt_d.ap().rearrange("ct p (g t) -> g p ct t", t=GT)
        yv = y_d.ap().rearrange("(g tt p) c -> g p tt c", tt=NTT, p=128)

        # Tiles live across the gen(g+1) / attention(g) software pipeline.
        st = {}

        def gen_chunks(g):
            """QKV generation for group g as a list of emit-closures."""
            XT_sb = xt.tile([128, 3, GT], BF16, name=f"XT_{g}", tag="XT")
            QKT_sb = qkt.tile([128, 6, GT], BF16, name=f"QKT_{g}", tag="QKT")
            V65_sb = v65.tile([128, NTT, H * 65], BF16, name=f"V65_{g}",
                              tag="V65")
            st[g] = (QKT_sb, V65_sb)
            chunks = [lambda: dma(XT_sb[:], xtv[g])]

            def qkgen(ft):
                ps_qk = mm_ps.tile([128, 512], F32, name=f"psqk_{g}_{ft}",
                                   tag="mm")
                for ct in range(3):
                    nc.tensor.matmul(
                        ps_qk[:],
                        wat_sb[:, ct, 128 * ft:128 * (ft + 1)],
                        XT_sb[:, ct, :],
                        start=(ct == 0),
                        stop=(ct == 2),
                    )
                bias = bq_sb[:, ft:ft + 1] if ft < 3 else bk_sb[:, ft - 3:ft - 2]
                # split the eviction between ACT and DVE so neither engine
                # eats the whole 512-col copy in one wave
                nc.scalar.activation(QKT_sb[:, ft, 0:256], ps_qk[:, 0:256],
                                     AF.Identity, bias=bias)
                nc.vector.tensor_scalar_add(QKT_sb[:, ft, 256:512],
                                            ps_qk[:, 256:512], bias)

            def vgen(tt):
                ps_v = mm_ps.tile([128, 384], F32, name=f"psv_{g}_{tt}",
                                  tag="mm")
                for ct in range(3):
                    nc.tensor.matmul(
                        ps_v[:],
                        XT_sb[:, ct, 128 * tt:128 * (tt + 1)],
                        wat_sb[:, ct, 2 * C:3 * C],
                        start=(ct == 0),
                        stop=(ct == 2),
                    )
                v_view = V65_sb[:, tt, :].rearrange("p (h w) -> p h w", h=H)
                if tt % 2 == 0:
                    nc.vector.tensor_copy(
                        v_view[:, :, 0:64],
                        ps_v[:].rearrange("p (h w) -> p h w", h=H),
                    )
                else:
                    nc.scalar.copy(
                        v_view[:, :, 0:64],
                        ps_v[:].rearrange("p (h w) -> p h w", h=H),
                    )
                nc.gpsimd.memset(v_view[:, :, 64:65], 1.0)

            from functools import partial
            chunks += [partial(qkgen, ft) for ft in range(6)]
            chunks += [partial(vgen, tt) for tt in range(NTT)]
            return chunks

        def att_chunks(g):
            """Attention + projection for group g as a list of emit-closures.

            The per-head work is stage-split (scores -> mask -> PV ->
            normalize -> transpose) and emitted in waves so the PE never
            sits directly behind an ACT exp or a DVE mask.
            """
            QKT_sb, V65_sb = st[g]
            OT_sb = ot.tile([128, 3, GT], BF16, name=f"OT_{g}", tag="OT")
            Y_sb = yb.tile([128, NTT, C], F32, name=f"Y_{g}", tag="Y")
            hs = {}

            def scores(bl, h):
                q0 = 256 * bl
                ft, r0 = h // 2, 64 * (h % 2)
                hp, c0 = h // 2, 512 * (h % 2)
                KT = QKT_sb[r0:r0 + 64, 3 + ft, :]
                QT = QKT_sb[r0:r0 + 64, ft, :]

                if h % 2 == 0:
                    # one 2-bank PSUM tile per head pair; head-odd scores
                    # start at col 512 so each head stays within a bank
                    hs[(bl, hp, "s2")] = s_ps.tile(
                        [128, 1024], F32, name=f"pss_{g}_{bl}_{hp}", tag="s")
                ps_s = hs[(bl, hp, "s2")]
                # cols 0:256 = k-tile0 x (q 0:256), one matmul (shared lhsT)
                nc.tensor.matmul(
                    ps_s[:, c0:c0 + 256],
                    KT[:, q0:q0 + 128],
                    QT[:, q0:q0 + 256],
                    start=True, stop=True,
                )
                # cols 256:384 = k-tile1 x (q 128:256)
                nc.tensor.matmul(
                    ps_s[:, c0 + 256:c0 + 384],
                    KT[:, q0 + 128:q0 + 256],
                    QT[:, q0 + 128:q0 + 256],
                    start=True, stop=True,
                )

            def exp2(bl, hp):
                # one ACT exp for the whole head pair (strided PSUM read)
                ps_s = hs.pop((bl, hp, "s2"))
                P2_sb = pp.tile([128, 2, 384], BF16, name=f"P_{g}_{bl}_{hp}",
                                tag="P")
                sv = ps_s[:, :].rearrange("p (a b) -> p a b", a=2)[:, :, 0:384]
                nc.scalar.activation(P2_sb[:], sv, AF.Exp)
                hs[(bl, hp, "P")] = P2_sb

            def maskp(bl, h):
                # zero the upper-triangular (k > q) entries of the two
                # diagonal quadrants (cols 0:128 and 256:384) with a single
                # strided 0/1 mask multiply on DVE
                P2_sb = hs[(bl, h // 2, "P")]
                pq = P2_sb[:, h % 2, :].rearrange(
                    "p (a b) -> p a b", a=3)[:, 0:3:2, :]
                nc.vector.tensor_tensor(pq, pq, trim2_sb[:], op=ALU.mult)

            def pv(bl, h):
                hp, c0 = h // 2, 130 * (h % 2)
                P_sb = hs[(bl, hp, "P")][:, h % 2, :]
                vt0 = V65_sb[:, 2 * bl, 65 * h:65 * h + 65]
                vt1 = V65_sb[:, 2 * bl + 1, 65 * h:65 * h + 65]
                # O[q, f] with q on partitions; col 64/129 = Z; both heads
                # of a pair share one PSUM tile
                if h % 2 == 0:
                    hs[(bl, hp, "o2")] = o_ps.tile(
                        [128, 260], F32, name=f"pso_{g}_{bl}_{hp}", tag="o")
                else:
                    hs.pop((bl, hp, "P"))
                ps_o = hs[(bl, hp, "o2")]
                nc.tensor.matmul(ps_o[:, c0:c0 + 65], P_sb[:, 0:128], vt0,
                                 start=True, stop=True)
                nc.tensor.matmul(ps_o[:, c0 + 65:c0 + 130], P_sb[:, 128:256],
                                 vt0, start=True, stop=False)
                nc.tensor.matmul(ps_o[:, c0 + 65:c0 + 130], P_sb[:, 256:384],
                                 vt1, start=False, stop=True)

            def norma(bl, hp):
                # one paired PSUM->SBUF eviction for both heads
                ps_o = hs.pop((bl, hp, "o2"))
                Oq_sb = oe.tile([128, 260], F32, name=f"Oq_{g}_{bl}_{hp}",
                                tag="Oq")
                if (bl * 3 + hp) % 2 == 0:
                    nc.vector.tensor_copy(Oq_sb[:], ps_o[:])
                else:
                    nc.scalar.copy(Oq_sb[:], ps_o[:])
                hs[(bl, hp, "q")] = Oq_sb

            def normb(bl, hp):
                Oq_sb = hs.pop((bl, hp, "q"))
                # per-partition (per-q) normalize on Pool into the head-PAIR
                # tiles: OnP_qt [128 q, 128] = [head-even f | head-odd f]
                OnP0 = onp.tile([128, 128], BF16, name=f"OnP0_{g}_{bl}_{hp}",
                                tag="OnP")
                OnP1 = onp.tile([128, 128], BF16, name=f"OnP1_{g}_{bl}_{hp}",
                                tag="OnP")
                for par in range(2):
                    b0 = 130 * par
                    fcol = 64 * par
                    nc.gpsimd.normalize_recip(
                        OnP0[:, fcol:fcol + 64], Oq_sb[:, b0:b0 + 64],
                        Oq_sb[:, b0 + 64:b0 + 65])
                    nc.gpsimd.normalize_recip(
                        OnP1[:, fcol:fcol + 64], Oq_sb[:, b0 + 65:b0 + 129],
                        Oq_sb[:, b0 + 129:b0 + 130])
                hs[(bl, hp, "np0")] = OnP0
                hs[(bl, hp, "np1")] = OnP1

            def otrp(bl, hp):
                # PE-transpose each pair tile straight into the projection
                # layout: OT[0:128, hp, qtile] = OnP_qt.T (f-pair rows); one
                # full-width copy per q-tile, split DVE / ACT
                q0 = 256 * bl
                for qt in range(2):
                    OnP = hs.pop((bl, hp, "np0" if qt == 0 else "np1"))
                    ps_t = o_ps.tile([128, 128], BF16,
                                     name=f"ptr_{g}_{bl}_{hp}_{qt}",
                                     tag="o")
                    nc.tensor.transpose(ps_t[:], OnP[:], identb_sb[:])
                    dst = OT_sb[:, hp, q0 + 128 * qt:q0 + 128 * (qt + 1)]
                    if qt == 0:
                        nc.vector.tensor_copy(dst, ps_t[:])
                    else:
                        nc.scalar.copy(dst, ps_t[:])

            def proj(tt):
                ps_y = mm_ps.tile([128, 384], F32, name=f"psy_{g}_{tt}",
                                  tag="mm")
                for hp in range(3):
                    nc.tensor.matmul(
                        ps_y[:],
                        OT_sb[:, hp, 128 * tt:128 * (tt + 1)],
                        wpt_sb[:, hp, :],
                        start=(hp == 0),
                        stop=(hp == 2),
                    )
                nc.vector.tensor_tensor(Y_sb[:, tt, :], ps_y[:], beff_sb[:],
                                        op=ALU.add)

            from functools import partial
            heads = [(bl, h) for bl in range(G) for h in range(H)]
            nh = len(heads)

            def pair_stage(fn, j, wave):
                if 0 <= j < nh and heads[j][1] % 2 == 1:
                    wave.append(partial(fn, heads[j][0], heads[j][1] // 2))

            chunks = []
            for i in range(nh + 8):
                # wave order puts the critical-path feeders first: the mask
                # leads the DVE queue (pv consumes it next wave), pv leads
                # the PE queue, exp leads the ACT queue
                wave = []
                if 0 <= i - 3 < nh:
                    wave.append(partial(maskp, *heads[i - 3]))
                if 0 <= i - 4 < nh:
                    wave.append(partial(pv, *heads[i - 4]))
                pair_stage(exp2, i - 1, wave)
                if i < nh:
                    wave.append(partial(scores, *heads[i]))
                pair_stage(norma, i - 5, wave)
                pair_stage(normb, i - 6, wave)
                pair_stage(otrp, i - 7, wave)
                j = i - 7
                if 0 <= j < nh and heads[j] == (heads[j][0], H - 1):
                    # this wave completed otrp(bl, 2): OT q-block bl is
                    # done -> project + ship it while later heads drain
                    bl = heads[j][0]
                    wave.append(partial(proj, 2 * bl))
                    wave.append(partial(proj, 2 * bl + 1))
                    wave.append(lambda b=bl: dma(
                        yv[g][:, 2 * b:2 * b + 2, :],
                        Y_sb[:, 2 * b:2 * b + 2, :]))
                chunks.append(lambda w=wave: [f() for f in w])
            return chunks

        # software pipeline: interleave gen(g+1) between attention(g) chunks
        for c in gen_chunks(0):
            c()
        for g in range(NG):
            att = att_chunks(g)
            gen = gen_chunks(g + 1) if g + 1 < NG else []
            n = max(len(att), len(gen))
            for i in range(n):
                if i < len(att):
                    att[i]()
                if i < len(gen):
                    gen[i]()


_CACHE = {}


def _build_nc():
    if "nc" in _CACHE:
        return _CACHE["nc"]
    nc = bacc.Bacc("TRN2", target_bir_lowering=False, debug=False,
                   num_devices=N_CORES)
    xt_d = nc.dram_tensor("xt", [3, 128, TOK], BF16, kind="ExternalInput")
    wat_d = nc.dram_tensor("w_attnT", [C, 3 * C], BF16, kind="ExternalInput")
    wpt_d = nc.dram_tensor("w_projT", [128, 3, C], BF16, kind="ExternalInput")
    bq_d = nc.dram_tensor("bq", [128, 3], F32, kind="ExternalInput")
    bk_d = nc.dram_tensor("bk", [128, 3], F32, kind="ExternalInput")
    beff_d = nc.dram_tensor("beff", [128, C], F32, kind="ExternalInput")
    trim_d = nc.dram_tensor("trim", [128, 2, 128], BF16, kind="ExternalInput")
    identb_d = nc.dram_tensor("identb", [128, 128], BF16, kind="ExternalInput")
    y_d = nc.dram_tensor("y", [TOK, C], F32, kind="ExternalOutput")

    with tile.TileContext(nc) as tc:
        _body(tc, xt_d, wat_d, wpt_d, bq_d, bk_d, beff_d, trim_d, identb_d,
              y_d)
    nc.compile()
    _CACHE["nc"] = nc
    return nc


def _host_inputs(x, w_attn, b_attn, w_proj, b_proj):
    """Build the per-core input maps (host-side prep of weights/constants)."""
    bf16 = ml_dtypes.bfloat16
    w_attnT = np.ascontiguousarray(w_attn.T).astype(np.float32)    # [C, 3C]
    w_attnT[:, :C] *= 0.125          # fold score scale into Q weights
    # w_projT per head pair: wpt[p, hp, of] = w_proj[of, 128*hp + p]
    wpt = np.ascontiguousarray(
        w_proj.T.reshape(3, 128, C).transpose(1, 0, 2))
    bq = np.ascontiguousarray((0.125 * b_attn[:C]).reshape(3, 128).T)
    bk = np.ascontiguousarray(b_attn[C:2 * C].reshape(3, 128).T)
    b_eff = np.broadcast_to(b_proj + w_proj @ b_attn[2 * C:], (128, C))

    # 0/1 lower-triangular (k <= q) mask, doubled for the two diagonal
    # quadrants masked in one strided op
    p = np.arange(128)[:, None]
    j = np.arange(128)[None, :]
    mask = (p <= j).astype(np.float32)
    mask2 = np.stack([mask, mask], axis=1)          # [128, 2, 128]

    common = {
        "w_attnT": w_attnT.astype(bf16),
        "w_projT": wpt.astype(bf16),
        "bq": bq.astype(np.float32),
        "bk": bk.astype(np.float32),
        "beff": np.ascontiguousarray(b_eff).astype(np.float32),
        "trim": np.ascontiguousarray(mask2).astype(bf16),
        "identb": np.eye(128, dtype=np.float32).astype(bf16),
    }
    # host-side transpose: x [B,T,C] -> per-core XT [3, 128, TOK]
    xs = x.reshape(N_CORES, TOK, C)
    in_maps = []
    for c in range(N_CORES):
        xt_c = np.ascontiguousarray(xs[c].T.reshape(3, 128, TOK))
        m = dict(common)
        m["xt"] = xt_c.astype(bf16)
        in_maps.append(m)
    return in_maps


def kernel(x, w_attn, b_attn, w_proj, b_proj):
    from concourse.bass_utils import run_bass_kernel_spmd

    x = np.asarray(x, dtype=np.float32)
    w_attn = np.asarray(w_attn, dtype=np.float32)
    b_attn = np.asarray(b_attn, dtype=np.float32)
    w_proj = np.asarray(w_proj, dtype=np.float32)
    b_proj = np.asarray(b_proj, dtype=np.float32)

    nc = _build_nc()
    in_maps = _host_inputs(x, w_attn, b_attn, w_proj, b_proj)
    res = run_bass_kernel_spmd(nc, in_maps, core_ids=list(range(N_CORES)))
    y = np.stack([res.results[c]["y"] for c in range(N_CORES)])
    return y.reshape(B, T, C)
